# revision 37
# baseline (speedup 1.0000x reference)
"""Trainium2 Bass kernel: per-pixel 5x5-patch channel covariance.

R[b,h,w,k,l] = (1/N) sum_n (p_kn - mu_k)(p_ln - mu_l)   (N=25, reflect pad)

Identity:  R = box5x5(S_k * S_l)/25 - mu_k * mu_l,  mu = box5x5(S)/25.

Device computes ONLY the 136 upper-triangle pair channels box5x5(S_k*S_l)/25
(host pre-scales S by 1/5 so two weight-1 banded box passes give /25).
Host computes mu (cheap separable box in fp32), subtracts mu_k*mu_l, and
mirrors the symmetric lower triangle -- all trivially vectorized numpy.

Per core (shard = one batch x one H-half): products for shard rows 0..127
are computed once on a full 128-partition tile (DVE/Pool); the 4 halo rows
128..131 only feed h-outs 124..127, so their products are precomputed on
the host (tiny) and folded in via an N=4 accumulating matmul.

  products (DVE 2x / Pool)
    -> stage-1 row-box banded matmuls (TensorE, psum [w, (8c,2chunk,64h)])
    -> psum->sbuf copies into i1[w, ch, chunk, h] (Act/DVE)
    -> stage-2 col-box matmuls lhsT=i1[:,c,k,:], rhs=BW chunk (psum [h, w])
    -> psum->sbuf copies (Act/DVE)
    -> DMA out channel-major [136, 128, 256] (512B descriptors)

Sharding: 8 cores = 4 batches x 2 H-halves.  Fully data parallel.
"""
import sys

sys.path.insert(0, "/opt/trn_rl_repo")

from contextlib import ExitStack

import numpy as np

import concourse.bacc as bacc
import concourse.mybir as mybir
import concourse.tile as tile
from concourse import bass_utils

B, K, H, W = 4, 16, 256, 256
HH = 128           # output rows per core
SR = 132           # shard rows (128 + 2 halo each side, edge-clamped)
NPAIR = K * (K + 1) // 2   # 136 upper-triangle channels
NOCT = NPAIR // 8          # 17 channel octets
F32 = mybir.dt.float32
BF16 = mybir.dt.bfloat16

# Pool (GPSIMD) cannot read PSUM, so psum->sbuf copies go Act/DVE; Pool
# takes these product octets (emitted as a pre-pass, consumption order).
POOL_OCTS = [1, 3, 5, 7, 9, 11, 13, 15]


def _reflect_idx(i, n):
    if i < 0:
        return -i
    if i >= n:
        return 2 * (n - 1) - i
    return i


def _build_bw():
    """[128, 512] col-box weights, reflect folded: [:, c*256:(c+1)*256] =
    M[c*128:(c+1)*128, :] where M[w_src, w_out] is the 256x256 band."""
    M = np.zeros((W, W), dtype=np.float32)
    for w in range(W):
        for j in range(5):
            M[_reflect_idx(w - 2 + j, W), w] += 1.0
    out = np.zeros((128, 512), dtype=np.float32)
    out[:, 0:256] = M[0:128, :]
    out[:, 256:512] = M[128:256, :]
    return out


def _build_br(half):
    """[68, 128] row-box weights: cols rt*64+hl; rows shard-local within rt."""
    hbase = half * HH
    M = np.zeros((68, 128), dtype=np.float32)
    for rt in range(2):
        for hl in range(64):
            hg = hbase + rt * 64 + hl
            for i in range(5):
                r = _reflect_idx(hg - 2 + i, H)
                j = r + 2 - hbase          # canonical shard row
                M[j - rt * 64, rt * 64 + hl] += 1.0
    return M


def _ksegs_in_octet(oct_idx):
    """Pair channels 0..135 in (k outer, l=k..15) order. For channel octet
    [oct*8, oct*8+8) return (j0, k, l0, nl): local offset, k, first l, count."""
    lo, hi = oct_idx * 8, oct_idx * 8 + 8
    segs = []
    p = 0
    for k in range(K):
        n = K - k
        s, e = p, p + n
        a, b = max(lo, s), min(hi, e)
        if a < b:
            segs.append((a - lo, k, k + (a - s), b - a))
        p += n
    return segs


def _build_kernel(pool_octs=None, dve_sched=(5, (1, 3)),
                  ps1_bufs=2, ps2_bufs=2, r_bufs=4, t_bufs=8):
    if pool_octs is None:
        pool_octs = POOL_OCTS
    nc = bacc.Bacc("TRN2", target_bir_lowering=False, debug=False)
    S_d = nc.dram_tensor("S", [HH, K, W], BF16, kind="ExternalInput").ap()
    P4_d = nc.dram_tensor("P4", [NPAIR, 8, W], BF16, kind="ExternalInput").ap()
    BR_d = nc.dram_tensor("BR", [68, 128], BF16, kind="ExternalInput").ap()
    BW_d = nc.dram_tensor("BW", [128, 512], BF16, kind="ExternalInput").ap()
    R_d = nc.dram_tensor("R", [NPAIR, HH, W], BF16, kind="ExternalOutput").ap()

    with tile.TileContext(nc) as tc, ExitStack() as ctx:
        const_p = ctx.enter_context(tc.tile_pool(name="const", bufs=1))
        sp_p = ctx.enter_context(tc.tile_pool(name="sp", bufs=1))
        t_p = ctx.enter_context(tc.tile_pool(name="tprod", bufs=t_bufs))
        tp_p = ctx.enter_context(tc.tile_pool(name="tpool", bufs=1))
        i1_p = ctx.enter_context(tc.tile_pool(name="i1", bufs=1))
        r_p = ctx.enter_context(tc.tile_pool(name="rout", bufs=r_bufs))
        ps1_p = ctx.enter_context(
            tc.tile_pool(name="ps1", bufs=ps1_bufs, space="PSUM"))
        ps2_p = ctx.enter_context(
            tc.tile_pool(name="ps2", bufs=ps2_bufs, space="PSUM"))

        ncopy = 0

        def copy_psum(dst, src):
            nonlocal ncopy
            if ncopy % dve_sched[0] in dve_sched[1]:
                nc.vector.tensor_copy(dst, src)
            else:
                nc.scalar.copy(dst, src)
            ncopy += 1

        br = const_p.tile([68, 128], BF16)
        br_mid = const_p.tile([128, 60], BF16)  # BR rows 0..59 at parts 64..127
        br8 = const_p.tile([8, 4], BF16)        # tail band: rows 124..131
        bw = const_p.tile([128, 512], BF16)
        nc.sync.dma_start(br[:], BR_d)
        nc.sync.dma_start(br_mid[64:128, :], BR_d[0:64, 64:124])
        nc.sync.dma_start(br8[:], BR_d[60:68, 124:128])
        nc.sync.dma_start(bw[:], BW_d)

        sp = sp_p.tile([HH, K, W], BF16)
        p4_p = ctx.enter_context(tc.tile_pool(name="p4", bufs=6))
        # split loads so the first product octets can start sooner
        nc.sync.dma_start(sp[:, 0:8, :], S_d[:, 0:8, :])
        nc.sync.dma_start(sp[:, 8:16, :], S_d[:, 8:16, :])

        # i1[w_local, ch, chunk, h]: stage-1 output; chunk X covers w 128X..+128
        i1 = i1_p.tile([128, NPAIR, 2, HH], BF16, name="i1")

        def products(oc, mul, pool):
            T = pool.tile([HH, 8, W], BF16,
                          name=f"T{oc}" if pool is tp_p else "T")
            for (j0, k, l0, nl) in _ksegs_in_octet(oc):
                in0 = sp[:, k, :].unsqueeze(1).broadcast_to([HH, nl, W])
                mul(T[:, j0:j0 + nl, :], in0, sp[:, l0:l0 + nl, :])
            return T

        # Pool pre-pass: spread product octets, in consumption order
        prod_T = {}
        for oc in pool_octs:
            prod_T[oc] = products(oc, nc.gpsimd.tensor_mul, tp_p)

        p4_T = {}

        def prefetch_products(ocp):
            """Emit DVE products and tail-product DMAs for group ocp ahead of
            older copies in the DVE queue (products gate PE)."""
            for oi in range(2 if ocp < 8 else 1):
                oc = ocp * 2 + oi
                if oc >= NOCT:
                    continue
                p4o = p4_p.tile([8, 8, W], BF16, name="p4o")
                nc.sync.dma_start(p4o[:], P4_d[oc * 8:(oc + 1) * 8])
                p4_T[oc] = p4o
                if oc not in prod_T:
                    prod_T[oc] = products(oc, nc.vector.tensor_mul, t_p)

        bw0 = bw[:, 0:256]
        bw1 = bw[:, 256:512]
        brt0 = br[:, 0:64]
        br_midv = br_mid[64:128, :]
        prefetch_products(0)
        for ocp in range(9):            # 16-channel groups (last is 8)
            nocts = 2 if ocp < 8 else 1
            if ocp < 8:
                prefetch_products(ocp + 1)
            # ---- stage 1: row box for this group's octets, both row-tiles
            for oi in range(nocts):
                oc = ocp * 2 + oi
                T = prod_T[oc]
                p4o = p4_T[oc]
                for rt in range(2):
                    ps1 = ps1_p.tile([128, 1024], F32, name="ps1")
                    for j in range(8):
                        for c in range(2):
                            o = j * 128 + c * 64
                            wsl = slice(c * 128, (c + 1) * 128)
                            if rt == 0:
                                nc.tensor.matmul(ps1[:, o:o + 64],
                                                 T[0:68, j, wsl], brt0,
                                                 start=True, stop=True)
                            else:
                                # outs 64..123 from product rows 64..127,
                                # outs 124..127 from host tail products
                                nc.tensor.matmul(ps1[:, o:o + 60],
                                                 T[64:128, j, wsl], br_midv,
                                                 start=True, stop=True)
                                nc.tensor.matmul(ps1[:, o + 60:o + 64],
                                                 p4o[:, j, wsl], br8[:],
                                                 start=True, stop=True)
                    copy_psum(
                        i1[:, oc * 8:(oc + 1) * 8, :, rt * 64:(rt + 1) * 64],
                        ps1[:].rearrange("p (c k h) -> p c k h", c=8, k=2))
            # ---- stage 2: col box -> [128 h, 256 w] per channel
            c0, nch = ocp * 16, 8 * nocts
            for cq in range(nch // 4):
                ps2 = ps2_p.tile([128, 1024], F32, name="ps2")
                for ci in range(4):
                    c = c0 + cq * 4 + ci
                    nc.tensor.matmul(ps2[:, ci * 256:(ci + 1) * 256],
                                     i1[:, c, 0, :], bw0,
                                     start=True, stop=False)
                    nc.tensor.matmul(ps2[:, ci * 256:(ci + 1) * 256],
                                     i1[:, c, 1, :], bw1,
                                     start=False, stop=True)
                rsb = r_p.tile([128, 4, W], BF16, name="rsb")
                copy_psum(rsb[:],
                          ps2[:].rearrange("p (c w) -> p c w", c=4))
                cb = c0 + cq * 4
                dview = R_d[cb:cb + 4, :, :].transpose([1, 0, 2])
                nc.sync.dma_start(dview, rsb[:])

    nc.compile()
    return nc


_NC_CACHE = {}


def _get_nc():
    if "nc" not in _NC_CACHE:
        _NC_CACHE["nc"] = _build_kernel()
    return _NC_CACHE["nc"]


def _prep_in_maps(S):
    S = np.asarray(S, dtype=np.float32)
    np_bf16 = mybir.dt.np(BF16)
    bw = _build_bw().astype(np_bf16)
    brs = [(_build_br(h)).astype(np_bf16) for h in range(2)]
    iu, il = np.triu_indices(K)
    Ss = S * np.float32(0.2)
    in_maps = []
    for b in range(B):
        for half in range(2):
            hbase = half * HH
            rows = np.clip(np.arange(hbase - 2, hbase + 130), 0, H - 1)
            shard = Ss[b][:, rows, :]                      # [K, 132, 256]
            s128 = np.ascontiguousarray(
                shard[:, 0:HH, :].transpose(1, 0, 2)).astype(np_bf16)
            # host products for tail rows 124..131 (feed h-outs 124..127),
            # laid out [(oct, r), ch_in_oct, w] to match the device tiles
            tail = shard[:, 124:SR, :]                     # [K, 8, 256]
            p4 = (tail[iu] * tail[il]).reshape(NOCT, 8, 8, W)
            p4 = np.ascontiguousarray(
                p4.transpose(0, 2, 1, 3).reshape(NPAIR, 8, W)).astype(np_bf16)
            in_maps.append({"S": s128, "P4": p4, "BR": brs[half], "BW": bw})
    return in_maps


def _box25(x):
    """Separable 5x5 box sum with reflect padding over last two axes."""
    xp = np.pad(x, ((0, 0), (0, 0), (2, 2), (2, 2)), mode="reflect")
    yh = xp[:, :, 0:H, :].copy()
    for i in range(1, 5):
        yh += xp[:, :, i:i + H, :]
    y = yh[:, :, :, 0:W].copy()
    for j in range(1, 5):
        y += yh[:, :, :, j:j + W]
    return y


def _assemble(results, S):
    iu, il = np.triu_indices(K)            # same order as device channels
    mu = _box25(np.asarray(S, np.float32)) * np.float32(1.0 / 25.0)
    out = np.empty((B, H, W, K, K), dtype=np.float32)
    for i in range(8):
        b, half = divmod(i, 2)
        hs = slice(half * HH, (half + 1) * HH)
        r = np.asarray(results[i]["R"]).astype(np.float32)   # [136, 128, 256]
        v = r - mu[b, iu, hs, :] * mu[b, il, hs, :]          # [136, 128, 256]
        v = np.moveaxis(v, 0, -1)                            # [128, 256, 136]
        flat = np.empty((HH, W, K * K), dtype=np.float32)
        flat[..., iu * K + il] = v
        flat[..., il * K + iu] = v
        out[b, hs] = flat.reshape(HH, W, K, K)
    return out


def kernel(S):
    """S: [4, 16, 256, 256] float32 -> R: [4, 256, 256, 16, 16] float32."""
    nc = _get_nc()
    in_maps = _prep_in_maps(S)
    res = bass_utils.run_bass_kernel_spmd(nc, in_maps, list(range(8)))
    return _assemble(res.results, S)


# revision 52
# speedup vs baseline: 1.0818x; 1.0818x over previous
"""Trainium2 Bass kernel: per-pixel 5x5-patch channel covariance.

R[b,h,w,k,l] = (1/N) sum_n (p_kn - mu_k)(p_ln - mu_l)   (N=25, reflect pad)

Identity:  R = box5x5(S_k * S_l)/25 - mu_k * mu_l,  mu = box5x5(S)/25.

Device computes ONLY the 136 upper-triangle pair channels box5x5(S_k*S_l)/25
(host pre-scales S by 1/5 so two weight-1 banded box passes give /25).
Host computes mu (cheap separable box in fp32), subtracts mu_k*mu_l, and
mirrors the symmetric lower triangle -- all trivially vectorized numpy.

Per core (shard = one batch x one H-half): products for shard rows 0..127
are computed once on a full 128-partition tile (DVE/Pool); the 4 halo rows
128..131 only feed h-outs 124..127, so their products are precomputed on
the host (tiny) and folded in via an N=4 accumulating matmul.

  products (DVE 2x / Pool)
    -> stage-1 row-box banded matmuls (TensorE, psum [w, (8c,2chunk,64h)])
    -> psum->sbuf copies into i1[w, ch, chunk, h] (Act/DVE)
    -> stage-2 col-box matmuls lhsT=i1[:,c,k,:], rhs=BW chunk (psum [h, w])
    -> psum->sbuf copies (Act/DVE)
    -> DMA out channel-major [136, 128, 256] (512B descriptors)

Sharding: 8 cores = 4 batches x 2 H-halves.  Fully data parallel.
"""
import sys

sys.path.insert(0, "/opt/trn_rl_repo")

from contextlib import ExitStack

import numpy as np

import concourse.bacc as bacc
import concourse.mybir as mybir
import concourse.tile as tile
from concourse import bass_utils

B, K, H, W = 4, 16, 256, 256
HH = 128           # output rows per core
SR = 132           # shard rows (128 + 2 halo each side, edge-clamped)
NPAIR = K * (K + 1) // 2   # 136 upper-triangle channels
NOCT = NPAIR // 8          # 17 channel octets
F32 = mybir.dt.float32
BF16 = mybir.dt.bfloat16

# Pool (GPSIMD) cannot read PSUM, so psum->sbuf copies go Act/DVE; Pool
# takes these product octets (emitted as a pre-pass, consumption order).
POOL_OCTS = [1, 3, 5, 7, 9, 11, 13, 15]


def _reflect_idx(i, n):
    if i < 0:
        return -i
    if i >= n:
        return 2 * (n - 1) - i
    return i


def _build_bw():
    """[128, 512] col-box weights, reflect folded: [:, c*256:(c+1)*256] =
    M[c*128:(c+1)*128, :] where M[w_src, w_out] is the 256x256 band."""
    M = np.zeros((W, W), dtype=np.float32)
    for w in range(W):
        for j in range(5):
            M[_reflect_idx(w - 2 + j, W), w] += 1.0
    out = np.zeros((128, 512), dtype=np.float32)
    out[:, 0:256] = M[0:128, :]
    out[:, 256:512] = M[128:256, :]
    return out


def _build_br(half):
    """[68, 128] row-box weights: cols rt*64+hl; rows shard-local within rt."""
    hbase = half * HH
    M = np.zeros((68, 128), dtype=np.float32)
    for rt in range(2):
        for hl in range(64):
            hg = hbase + rt * 64 + hl
            for i in range(5):
                r = _reflect_idx(hg - 2 + i, H)
                j = r + 2 - hbase          # canonical shard row
                M[j - rt * 64, rt * 64 + hl] += 1.0
    return M


def _ksegs_in_octet(oct_idx):
    """Pair channels 0..135 in (k outer, l=k..15) order. For channel octet
    [oct*8, oct*8+8) return (j0, k, l0, nl): local offset, k, first l, count."""
    lo, hi = oct_idx * 8, oct_idx * 8 + 8
    segs = []
    p = 0
    for k in range(K):
        n = K - k
        s, e = p, p + n
        a, b = max(lo, s), min(hi, e)
        if a < b:
            segs.append((a - lo, k, k + (a - s), b - a))
        p += n
    return segs


def _build_kernel(pool_octs=None, dve_sched=(5, (1, 3)),
                  ps1_bufs=2, ps2_bufs=2, r_bufs=4, t_bufs=8, s2_grain=4,
                  split_copies=False, s2_lag=1):
    if pool_octs is None:
        pool_octs = POOL_OCTS
    nc = bacc.Bacc("TRN2", target_bir_lowering=False, debug=False)
    S_d = nc.dram_tensor("S", [HH, K, W], BF16, kind="ExternalInput").ap()
    P4_d = nc.dram_tensor("P4", [NPAIR, 8, W], BF16, kind="ExternalInput").ap()
    BR_d = nc.dram_tensor("BR", [68, 128], BF16, kind="ExternalInput").ap()
    BW_d = nc.dram_tensor("BW", [128, 512], BF16, kind="ExternalInput").ap()
    R_d = nc.dram_tensor("R", [NPAIR, HH, W], BF16, kind="ExternalOutput").ap()

    with tile.TileContext(nc) as tc, ExitStack() as ctx:
        const_p = ctx.enter_context(tc.tile_pool(name="const", bufs=1))
        sp_p = ctx.enter_context(tc.tile_pool(name="sp", bufs=1))
        t_p = ctx.enter_context(tc.tile_pool(name="tprod", bufs=t_bufs))
        tp_p = ctx.enter_context(tc.tile_pool(name="tpool", bufs=1))
        i1_p = ctx.enter_context(tc.tile_pool(name="i1", bufs=1))
        r_p = ctx.enter_context(tc.tile_pool(name="rout", bufs=r_bufs))
        ps1_p = ctx.enter_context(
            tc.tile_pool(name="ps1", bufs=ps1_bufs, space="PSUM"))
        ps2_p = ctx.enter_context(
            tc.tile_pool(name="ps2", bufs=ps2_bufs, space="PSUM"))

        ncopy = 0

        def copy_psum(dst, src):
            nonlocal ncopy
            if split_copies:
                # halve latency: two engines copy half each
                h = src.shape[1] // 2
                if ncopy % 2 == 0:
                    nc.vector.tensor_copy(dst[:, 0:h], src[:, 0:h])
                    nc.scalar.copy(dst[:, h:], src[:, h:])
                else:
                    nc.scalar.copy(dst[:, 0:h], src[:, 0:h])
                    nc.vector.tensor_copy(dst[:, h:], src[:, h:])
                ncopy += 1
                return
            if ncopy % dve_sched[0] in dve_sched[1]:
                nc.vector.tensor_copy(dst, src)
            else:
                nc.scalar.copy(dst, src)
            ncopy += 1

        br = const_p.tile([68, 128], BF16)
        br_mid = const_p.tile([128, 60], BF16)  # BR rows 0..59 at parts 64..127
        br8 = const_p.tile([8, 4], BF16)        # tail band: rows 124..131
        bw = const_p.tile([128, 512], BF16)
        sp = sp_p.tile([HH, K, W], BF16)
        p4_p = ctx.enter_context(tc.tile_pool(name="p4", bufs=6))
        nc.sync.dma_start(br[:], BR_d)
        nc.sync.dma_start(br_mid[64:128, :], BR_d[0:64, 64:124])
        nc.sync.dma_start(br8[:], BR_d[60:68, 124:128])
        nc.sync.dma_start(bw[:], BW_d)
        nc.sync.dma_start(sp[:, 0:8, :], S_d[:, 0:8, :])
        nc.sync.dma_start(sp[:, 8:16, :], S_d[:, 8:16, :])


        # i1[w_local, ch, chunk, h]: stage-1 output; chunk X covers w 128X..+128
        i1 = i1_p.tile([128, NPAIR, 2, HH], BF16, name="i1")

        def products(oc, mul, pool):
            T = pool.tile([HH, 8, W], BF16,
                          name=f"T{oc}" if pool is tp_p else "T")
            for (j0, k, l0, nl) in _ksegs_in_octet(oc):
                in0 = sp[:, k, :].unsqueeze(1).broadcast_to([HH, nl, W])
                mul(T[:, j0:j0 + nl, :], in0, sp[:, l0:l0 + nl, :])
            return T

        # Pool pre-pass: spread product octets, in consumption order
        prod_T = {}
        for oc in pool_octs:
            prod_T[oc] = products(oc, nc.gpsimd.tensor_mul, tp_p)

        p4_T = {}

        def prefetch_products(ocp):
            """Emit DVE products and tail-product DMAs for group ocp ahead of
            older copies in the DVE queue (products gate PE)."""
            for oi in range(2 if ocp < 8 else 1):
                oc = ocp * 2 + oi
                if oc >= NOCT:
                    continue
                p4o = p4_p.tile([8, 8, W], BF16, name="p4o")
                nc.sync.dma_start(p4o[:], P4_d[oc * 8:(oc + 1) * 8])
                p4_T[oc] = p4o
                if oc not in prod_T:
                    prod_T[oc] = products(oc, nc.vector.tensor_mul, t_p)

        bw0 = bw[:, 0:256]
        bw1 = bw[:, 256:512]
        brt0 = br[:, 0:64]
        br_midv = br_mid[64:128, :]

        def stage1(ocp):
            nocts = 2 if ocp < 8 else 1
            for oi in range(nocts):
                oc = ocp * 2 + oi
                T = prod_T[oc]
                p4o = p4_T[oc]
                for rt in range(2):
                    ps1 = ps1_p.tile([128, 1024], F32, name="ps1")
                    for j in range(8):
                        for c in range(2):
                            o = j * 128 + c * 64
                            wsl = slice(c * 128, (c + 1) * 128)
                            if rt == 0:
                                nc.tensor.matmul(ps1[:, o:o + 64],
                                                 T[0:68, j, wsl], brt0,
                                                 start=True, stop=True)
                            else:
                                # outs 64..123 from product rows 64..127,
                                # outs 124..127 from host tail products
                                nc.tensor.matmul(ps1[:, o:o + 60],
                                                 T[64:128, j, wsl], br_midv,
                                                 start=True, stop=True)
                                nc.tensor.matmul(ps1[:, o + 60:o + 64],
                                                 p4o[:, j, wsl], br8[:],
                                                 start=True, stop=True)
                    copy_psum(
                        i1[:, oc * 8:(oc + 1) * 8, :, rt * 64:(rt + 1) * 64],
                        ps1[:].rearrange("p (c k h) -> p c k h", c=8, k=2))

        def stage2(ocp):
            # ---- stage 2: col box -> [128 h, 256 w] per channel
            nocts = 2 if ocp < 8 else 1
            c0, nch = ocp * 16, 8 * nocts
            g = s2_grain
            for cq in range(nch // g):
                ps2 = ps2_p.tile([128, 256 * g], F32, name="ps2")
                for ci in range(g):
                    c = c0 + cq * g + ci
                    nc.tensor.matmul(ps2[:, ci * 256:(ci + 1) * 256],
                                     i1[:, c, 0, :], bw0,
                                     start=True, stop=False)
                    nc.tensor.matmul(ps2[:, ci * 256:(ci + 1) * 256],
                                     i1[:, c, 1, :], bw1,
                                     start=False, stop=True)
                rsb = r_p.tile([128, g, W], BF16, name="rsb")
                copy_psum(rsb[:],
                          ps2[:].rearrange("p (c w) -> p c w", c=g))
                cb = c0 + cq * g
                dview = R_d[cb:cb + g, :, :].transpose([1, 0, 2])
                nc.sync.dma_start(dview, rsb[:])

        # software-pipeline: keep PE s2_lag groups ahead of stage 2 so
        # stage-2 matmuls never head-block PE while their i1 copies land.
        for ocp in range(s2_lag + 1):
            prefetch_products(ocp)
        for ocp in range(s2_lag):
            stage1(ocp)
        for ocp in range(s2_lag, 9):
            prefetch_products(ocp + 1)
            stage1(ocp)
            stage2(ocp - s2_lag)
        for ocp in range(9 - s2_lag, 9):
            stage2(ocp)

    nc.compile()
    return nc


_NC_CACHE = {}


def _get_nc():
    if "nc" not in _NC_CACHE:
        _NC_CACHE["nc"] = _build_kernel()
    return _NC_CACHE["nc"]


def _prep_in_maps(S):
    S = np.asarray(S, dtype=np.float32)
    np_bf16 = mybir.dt.np(BF16)
    bw = _build_bw().astype(np_bf16)
    brs = [(_build_br(h)).astype(np_bf16) for h in range(2)]
    iu, il = np.triu_indices(K)
    Ss = S * np.float32(0.2)
    in_maps = []
    for b in range(B):
        for half in range(2):
            hbase = half * HH
            rows = np.clip(np.arange(hbase - 2, hbase + 130), 0, H - 1)
            shard = Ss[b][:, rows, :]                      # [K, 132, 256]
            s128 = np.ascontiguousarray(
                shard[:, 0:HH, :].transpose(1, 0, 2)).astype(np_bf16)
            # host products for tail rows 124..131 (feed h-outs 124..127),
            # laid out [(oct, r), ch_in_oct, w] to match the device tiles
            tail = shard[:, 124:SR, :]                     # [K, 8, 256]
            p4 = (tail[iu] * tail[il]).reshape(NOCT, 8, 8, W)
            p4 = np.ascontiguousarray(
                p4.transpose(0, 2, 1, 3).reshape(NPAIR, 8, W)).astype(np_bf16)
            in_maps.append({"S": s128, "P4": p4, "BR": brs[half], "BW": bw})
    return in_maps


def _box25(x):
    """Separable 5x5 box sum with reflect padding over last two axes."""
    xp = np.pad(x, ((0, 0), (0, 0), (2, 2), (2, 2)), mode="reflect")
    yh = xp[:, :, 0:H, :].copy()
    for i in range(1, 5):
        yh += xp[:, :, i:i + H, :]
    y = yh[:, :, :, 0:W].copy()
    for j in range(1, 5):
        y += yh[:, :, :, j:j + W]
    return y


def _assemble(results, S):
    iu, il = np.triu_indices(K)            # same order as device channels
    mu = _box25(np.asarray(S, np.float32)) * np.float32(1.0 / 25.0)
    out = np.empty((B, H, W, K, K), dtype=np.float32)
    for i in range(8):
        b, half = divmod(i, 2)
        hs = slice(half * HH, (half + 1) * HH)
        r = np.asarray(results[i]["R"]).astype(np.float32)   # [136, 128, 256]
        v = r - mu[b, iu, hs, :] * mu[b, il, hs, :]          # [136, 128, 256]
        v = np.moveaxis(v, 0, -1)                            # [128, 256, 136]
        flat = np.empty((HH, W, K * K), dtype=np.float32)
        flat[..., iu * K + il] = v
        flat[..., il * K + iu] = v
        out[b, hs] = flat.reshape(HH, W, K, K)
    return out


def kernel(S):
    """S: [4, 16, 256, 256] float32 -> R: [4, 256, 256, 16, 16] float32."""
    nc = _get_nc()
    in_maps = _prep_in_maps(S)
    res = bass_utils.run_bass_kernel_spmd(nc, in_maps, list(range(8)))
    return _assemble(res.results, S)


# revision 53
# speedup vs baseline: 1.1172x; 1.0327x over previous
"""Trainium2 Bass kernel: per-pixel 5x5-patch channel covariance.

R[b,h,w,k,l] = (1/N) sum_n (p_kn - mu_k)(p_ln - mu_l)   (N=25, reflect pad)

Identity:  R = box5x5(S_k * S_l)/25 - mu_k * mu_l,  mu = box5x5(S)/25.

Device computes ONLY the 136 upper-triangle pair channels box5x5(S_k*S_l)/25
(host pre-scales S by 1/5 so two weight-1 banded box passes give /25).
Host computes mu (cheap separable box in fp32), subtracts mu_k*mu_l, and
mirrors the symmetric lower triangle -- all trivially vectorized numpy.

Per core (shard = one batch x one H-half): products for shard rows 0..127
are computed once on a full 128-partition tile (DVE/Pool); the 4 halo rows
128..131 only feed h-outs 124..127, so their products are precomputed on
the host (tiny) and folded in via an N=4 accumulating matmul.

  products (DVE 2x / Pool)
    -> stage-1 row-box banded matmuls (TensorE, psum [w, (8c,2chunk,64h)])
    -> psum->sbuf copies into i1[w, ch, chunk, h] (Act/DVE)
    -> stage-2 col-box matmuls lhsT=i1[:,c,k,:], rhs=BW chunk (psum [h, w])
    -> psum->sbuf copies (Act/DVE)
    -> DMA out channel-major [136, 128, 256] (512B descriptors)

Sharding: 8 cores = 4 batches x 2 H-halves.  Fully data parallel.
"""
import sys

sys.path.insert(0, "/opt/trn_rl_repo")

from contextlib import ExitStack

import numpy as np

import concourse.bacc as bacc
import concourse.mybir as mybir
import concourse.tile as tile
from concourse import bass_utils

B, K, H, W = 4, 16, 256, 256
HH = 128           # output rows per core
SR = 132           # shard rows (128 + 2 halo each side, edge-clamped)
NPAIR = K * (K + 1) // 2   # 136 upper-triangle channels
NOCT = NPAIR // 8          # 17 channel octets
F32 = mybir.dt.float32
BF16 = mybir.dt.bfloat16

# Pool (GPSIMD) cannot read PSUM, so psum->sbuf copies go Act/DVE; Pool
# takes these product octets (emitted as a pre-pass, consumption order).
POOL_OCTS = [2, 4, 6, 8, 10, 12, 15]


def _reflect_idx(i, n):
    if i < 0:
        return -i
    if i >= n:
        return 2 * (n - 1) - i
    return i


def _build_bw():
    """[128, 512] col-box weights, reflect folded: [:, c*256:(c+1)*256] =
    M[c*128:(c+1)*128, :] where M[w_src, w_out] is the 256x256 band."""
    M = np.zeros((W, W), dtype=np.float32)
    for w in range(W):
        for j in range(5):
            M[_reflect_idx(w - 2 + j, W), w] += 1.0
    out = np.zeros((128, 512), dtype=np.float32)
    out[:, 0:256] = M[0:128, :]
    out[:, 256:512] = M[128:256, :]
    return out


def _build_br(half):
    """[68, 128] row-box weights: cols rt*64+hl; rows shard-local within rt."""
    hbase = half * HH
    M = np.zeros((68, 128), dtype=np.float32)
    for rt in range(2):
        for hl in range(64):
            hg = hbase + rt * 64 + hl
            for i in range(5):
                r = _reflect_idx(hg - 2 + i, H)
                j = r + 2 - hbase          # canonical shard row
                M[j - rt * 64, rt * 64 + hl] += 1.0
    return M


def _ksegs_in_octet(oct_idx):
    """Pair channels 0..135 in (k outer, l=k..15) order. For channel octet
    [oct*8, oct*8+8) return (j0, k, l0, nl): local offset, k, first l, count."""
    lo, hi = oct_idx * 8, oct_idx * 8 + 8
    segs = []
    p = 0
    for k in range(K):
        n = K - k
        s, e = p, p + n
        a, b = max(lo, s), min(hi, e)
        if a < b:
            segs.append((a - lo, k, k + (a - s), b - a))
        p += n
    return segs


def _build_kernel(pool_octs=None, dve_sched=(5, (1, 3)),
                  ps1_bufs=2, ps2_bufs=2, r_bufs=4, t_bufs=8, s2_grain=4,
                  split_copies=False, s2_lag=1):
    if pool_octs is None:
        pool_octs = POOL_OCTS
    nc = bacc.Bacc("TRN2", target_bir_lowering=False, debug=False)
    S_d = nc.dram_tensor("S", [HH, K, W], BF16, kind="ExternalInput").ap()
    P4_d = nc.dram_tensor("P4", [NPAIR, 8, W], BF16, kind="ExternalInput").ap()
    BR_d = nc.dram_tensor("BR", [68, 128], BF16, kind="ExternalInput").ap()
    BW_d = nc.dram_tensor("BW", [128, 512], BF16, kind="ExternalInput").ap()
    R_d = nc.dram_tensor("R", [NPAIR, HH, W], BF16, kind="ExternalOutput").ap()

    with tile.TileContext(nc) as tc, ExitStack() as ctx:
        const_p = ctx.enter_context(tc.tile_pool(name="const", bufs=1))
        sp_p = ctx.enter_context(tc.tile_pool(name="sp", bufs=1))
        t_p = ctx.enter_context(tc.tile_pool(name="tprod", bufs=t_bufs))
        tp_p = ctx.enter_context(tc.tile_pool(name="tpool", bufs=1))
        i1_p = ctx.enter_context(tc.tile_pool(name="i1", bufs=1))
        r_p = ctx.enter_context(tc.tile_pool(name="rout", bufs=r_bufs))
        ps1_p = ctx.enter_context(
            tc.tile_pool(name="ps1", bufs=ps1_bufs, space="PSUM"))
        ps2_p = ctx.enter_context(
            tc.tile_pool(name="ps2", bufs=ps2_bufs, space="PSUM"))

        ncopy = 0

        def copy_psum(dst, src):
            nonlocal ncopy
            if split_copies:
                # halve latency: two engines copy half each
                h = src.shape[1] // 2
                if ncopy % 2 == 0:
                    nc.vector.tensor_copy(dst[:, 0:h], src[:, 0:h])
                    nc.scalar.copy(dst[:, h:], src[:, h:])
                else:
                    nc.scalar.copy(dst[:, 0:h], src[:, 0:h])
                    nc.vector.tensor_copy(dst[:, h:], src[:, h:])
                ncopy += 1
                return
            if ncopy % dve_sched[0] in dve_sched[1]:
                nc.vector.tensor_copy(dst, src)
            else:
                nc.scalar.copy(dst, src)
            ncopy += 1

        br = const_p.tile([68, 128], BF16)
        br_mid = const_p.tile([128, 60], BF16)  # BR rows 0..59 at parts 64..127
        br8 = const_p.tile([8, 4], BF16)        # tail band: rows 124..131
        bw = const_p.tile([128, 512], BF16)
        sp = sp_p.tile([HH, K, W], BF16)
        p4_p = ctx.enter_context(tc.tile_pool(name="p4", bufs=6))
        nc.sync.dma_start(br[:], BR_d)
        nc.sync.dma_start(br_mid[64:128, :], BR_d[0:64, 64:124])
        nc.sync.dma_start(br8[:], BR_d[60:68, 124:128])
        nc.sync.dma_start(bw[:], BW_d)
        nc.sync.dma_start(sp[:, 0:8, :], S_d[:, 0:8, :])
        nc.sync.dma_start(sp[:, 8:16, :], S_d[:, 8:16, :])


        # i1[w_local, ch, chunk, h]: stage-1 output; chunk X covers w 128X..+128
        i1 = i1_p.tile([128, NPAIR, 2, HH], BF16, name="i1")

        def products(oc, mul, pool):
            T = pool.tile([HH, 8, W], BF16,
                          name=f"T{oc}" if pool is tp_p else "T")
            for (j0, k, l0, nl) in _ksegs_in_octet(oc):
                in0 = sp[:, k, :].unsqueeze(1).broadcast_to([HH, nl, W])
                mul(T[:, j0:j0 + nl, :], in0, sp[:, l0:l0 + nl, :])
            return T

        # Pool pre-pass: spread product octets, in consumption order
        prod_T = {}
        for oc in pool_octs:
            prod_T[oc] = products(oc, nc.gpsimd.tensor_mul, tp_p)

        p4_T = {}

        def prefetch_products(ocp):
            """Emit DVE products and tail-product DMAs for group ocp ahead of
            older copies in the DVE queue (products gate PE)."""
            for oi in range(2 if ocp < 8 else 1):
                oc = ocp * 2 + oi
                if oc >= NOCT:
                    continue
                p4o = p4_p.tile([8, 8, W], BF16, name="p4o")
                nc.sync.dma_start(p4o[:], P4_d[oc * 8:(oc + 1) * 8])
                p4_T[oc] = p4o
                if oc not in prod_T:
                    prod_T[oc] = products(oc, nc.vector.tensor_mul, t_p)

        bw0 = bw[:, 0:256]
        bw1 = bw[:, 256:512]
        brt0 = br[:, 0:64]
        br_midv = br_mid[64:128, :]

        def stage1(ocp):
            nocts = 2 if ocp < 8 else 1
            for oi in range(nocts):
                oc = ocp * 2 + oi
                T = prod_T[oc]
                p4o = p4_T[oc]
                for rt in range(2):
                    ps1 = ps1_p.tile([128, 1024], F32, name="ps1")
                    for j in range(8):
                        for c in range(2):
                            o = j * 128 + c * 64
                            wsl = slice(c * 128, (c + 1) * 128)
                            if rt == 0:
                                nc.tensor.matmul(ps1[:, o:o + 64],
                                                 T[0:68, j, wsl], brt0,
                                                 start=True, stop=True)
                            else:
                                # outs 64..123 from product rows 64..127,
                                # outs 124..127 from host tail products
                                nc.tensor.matmul(ps1[:, o:o + 60],
                                                 T[64:128, j, wsl], br_midv,
                                                 start=True, stop=True)
                                nc.tensor.matmul(ps1[:, o + 60:o + 64],
                                                 p4o[:, j, wsl], br8[:],
                                                 start=True, stop=True)
                    copy_psum(
                        i1[:, oc * 8:(oc + 1) * 8, :, rt * 64:(rt + 1) * 64],
                        ps1[:].rearrange("p (c k h) -> p c k h", c=8, k=2))

        def stage2(ocp):
            # ---- stage 2: col box -> [128 h, 256 w] per channel
            nocts = 2 if ocp < 8 else 1
            c0, nch = ocp * 16, 8 * nocts
            g = s2_grain
            for cq in range(nch // g):
                ps2 = ps2_p.tile([128, 256 * g], F32, name="ps2")
                for ci in range(g):
                    c = c0 + cq * g + ci
                    nc.tensor.matmul(ps2[:, ci * 256:(ci + 1) * 256],
                                     i1[:, c, 0, :], bw0,
                                     start=True, stop=False)
                    nc.tensor.matmul(ps2[:, ci * 256:(ci + 1) * 256],
                                     i1[:, c, 1, :], bw1,
                                     start=False, stop=True)
                rsb = r_p.tile([128, g, W], BF16, name="rsb")
                copy_psum(rsb[:],
                          ps2[:].rearrange("p (c w) -> p c w", c=g))
                cb = c0 + cq * g
                dview = R_d[cb:cb + g, :, :].transpose([1, 0, 2])
                nc.sync.dma_start(dview, rsb[:])

        # software-pipeline: keep PE s2_lag groups ahead of stage 2 so
        # stage-2 matmuls never head-block PE while their i1 copies land.
        for ocp in range(s2_lag + 1):
            prefetch_products(ocp)
        for ocp in range(s2_lag):
            stage1(ocp)
        for ocp in range(s2_lag, 9):
            prefetch_products(ocp + 1)
            stage1(ocp)
            stage2(ocp - s2_lag)
        for ocp in range(9 - s2_lag, 9):
            stage2(ocp)

    nc.compile()
    return nc


_NC_CACHE = {}


def _get_nc():
    if "nc" not in _NC_CACHE:
        _NC_CACHE["nc"] = _build_kernel()
    return _NC_CACHE["nc"]


def _prep_in_maps(S):
    S = np.asarray(S, dtype=np.float32)
    np_bf16 = mybir.dt.np(BF16)
    bw = _build_bw().astype(np_bf16)
    brs = [(_build_br(h)).astype(np_bf16) for h in range(2)]
    iu, il = np.triu_indices(K)
    Ss = S * np.float32(0.2)
    in_maps = []
    for b in range(B):
        for half in range(2):
            hbase = half * HH
            rows = np.clip(np.arange(hbase - 2, hbase + 130), 0, H - 1)
            shard = Ss[b][:, rows, :]                      # [K, 132, 256]
            s128 = np.ascontiguousarray(
                shard[:, 0:HH, :].transpose(1, 0, 2)).astype(np_bf16)
            # host products for tail rows 124..131 (feed h-outs 124..127),
            # laid out [(oct, r), ch_in_oct, w] to match the device tiles
            tail = shard[:, 124:SR, :]                     # [K, 8, 256]
            p4 = (tail[iu] * tail[il]).reshape(NOCT, 8, 8, W)
            p4 = np.ascontiguousarray(
                p4.transpose(0, 2, 1, 3).reshape(NPAIR, 8, W)).astype(np_bf16)
            in_maps.append({"S": s128, "P4": p4, "BR": brs[half], "BW": bw})
    return in_maps


def _box25(x):
    """Separable 5x5 box sum with reflect padding over last two axes."""
    xp = np.pad(x, ((0, 0), (0, 0), (2, 2), (2, 2)), mode="reflect")
    yh = xp[:, :, 0:H, :].copy()
    for i in range(1, 5):
        yh += xp[:, :, i:i + H, :]
    y = yh[:, :, :, 0:W].copy()
    for j in range(1, 5):
        y += yh[:, :, :, j:j + W]
    return y


def _assemble(results, S):
    iu, il = np.triu_indices(K)            # same order as device channels
    mu = _box25(np.asarray(S, np.float32)) * np.float32(1.0 / 25.0)
    out = np.empty((B, H, W, K, K), dtype=np.float32)
    for i in range(8):
        b, half = divmod(i, 2)
        hs = slice(half * HH, (half + 1) * HH)
        r = np.asarray(results[i]["R"]).astype(np.float32)   # [136, 128, 256]
        v = r - mu[b, iu, hs, :] * mu[b, il, hs, :]          # [136, 128, 256]
        v = np.moveaxis(v, 0, -1)                            # [128, 256, 136]
        flat = np.empty((HH, W, K * K), dtype=np.float32)
        flat[..., iu * K + il] = v
        flat[..., il * K + iu] = v
        out[b, hs] = flat.reshape(HH, W, K, K)
    return out


def kernel(S):
    """S: [4, 16, 256, 256] float32 -> R: [4, 256, 256, 16, 16] float32."""
    nc = _get_nc()
    in_maps = _prep_in_maps(S)
    res = bass_utils.run_bass_kernel_spmd(nc, in_maps, list(range(8)))
    return _assemble(res.results, S)


# revision 63
# speedup vs baseline: 1.1607x; 1.0389x over previous
"""Trainium2 Bass kernel: per-pixel 5x5-patch channel covariance.

R[b,h,w,k,l] = (1/N) sum_n (p_kn - mu_k)(p_ln - mu_l)   (N=25, reflect pad)

Identity:  R = box5x5(S_k * S_l)/25 - mu_k * mu_l,  mu = box5x5(S)/25.

Device computes ONLY the 136 upper-triangle pair channels box5x5(S_k*S_l)/25
(host pre-scales S by 1/5 so two weight-1 banded box passes give /25).
Host computes mu (cheap separable box in fp32), subtracts mu_k*mu_l, and
mirrors the symmetric lower triangle -- all trivially vectorized numpy.

Per core (shard = one batch x one H-half): products for shard rows 0..127
are computed once on a full 128-partition tile (DVE/Pool); the 4 halo rows
128..131 only feed h-outs 124..127, so their products are precomputed on
the host (tiny) and folded in via an N=4 accumulating matmul.

  products (DVE 2x / Pool)
    -> stage-1 row-box banded matmuls (TensorE, psum [w, (8c,2chunk,64h)])
    -> psum->sbuf copies into i1[w, ch, chunk, h] (Act/DVE)
    -> stage-2 col-box matmuls lhsT=i1[:,c,k,:], rhs=BW chunk (psum [h, w])
    -> psum->sbuf copies (Act/DVE)
    -> DMA out channel-major [136, 128, 256] (512B descriptors)

Sharding: 8 cores = 4 batches x 2 H-halves.  Fully data parallel.
"""
import sys

sys.path.insert(0, "/opt/trn_rl_repo")

from contextlib import ExitStack

import numpy as np

import concourse.bacc as bacc
import concourse.mybir as mybir
import concourse.tile as tile
from concourse import bass_utils

B, K, H, W = 4, 16, 256, 256
HH = 128           # output rows per core
SR = 132           # shard rows (128 + 2 halo each side, edge-clamped)
NPAIR = K * (K + 1) // 2   # 136 upper-triangle channels
NOCT = NPAIR // 8          # 17 channel octets
F32 = mybir.dt.float32
BF16 = mybir.dt.bfloat16

# Pool (GPSIMD) cannot read PSUM, so psum->sbuf copies go Act/DVE; Pool
# takes these product octets (emitted as a pre-pass, consumption order).
POOL_OCTS = [2, 4, 6, 8, 10, 12, 15]


def _reflect_idx(i, n):
    if i < 0:
        return -i
    if i >= n:
        return 2 * (n - 1) - i
    return i


def _build_bw():
    """[128, 512] col-box weights, reflect folded: [:, c*256:(c+1)*256] =
    M[c*128:(c+1)*128, :] where M[w_src, w_out] is the 256x256 band."""
    M = np.zeros((W, W), dtype=np.float32)
    for w in range(W):
        for j in range(5):
            M[_reflect_idx(w - 2 + j, W), w] += 1.0
    out = np.zeros((128, 512), dtype=np.float32)
    out[:, 0:256] = M[0:128, :]
    out[:, 256:512] = M[128:256, :]
    return out


def _build_br(half):
    """[68, 128] row-box weights: cols rt*64+hl; rows shard-local within rt."""
    hbase = half * HH
    M = np.zeros((68, 128), dtype=np.float32)
    for rt in range(2):
        for hl in range(64):
            hg = hbase + rt * 64 + hl
            for i in range(5):
                r = _reflect_idx(hg - 2 + i, H)
                j = r + 2 - hbase          # canonical shard row
                M[j - rt * 64, rt * 64 + hl] += 1.0
    return M


def _ksegs_in_octet(oct_idx):
    """Pair channels 0..135 in (k outer, l=k..15) order. For channel octet
    [oct*8, oct*8+8) return (j0, k, l0, nl): local offset, k, first l, count."""
    lo, hi = oct_idx * 8, oct_idx * 8 + 8
    segs = []
    p = 0
    for k in range(K):
        n = K - k
        s, e = p, p + n
        a, b = max(lo, s), min(hi, e)
        if a < b:
            segs.append((a - lo, k, k + (a - s), b - a))
        p += n
    return segs


def _build_kernel(pool_octs=None, dve_sched=(5, (1, 3)),
                  ps1_bufs=2, ps2_bufs=2, r_bufs=4, t_bufs=8, s2_grain=4,
                  split_copies=False, s2_lag=1, split_s2=False):
    if pool_octs is None:
        pool_octs = POOL_OCTS
    nc = bacc.Bacc("TRN2", target_bir_lowering=False, debug=False)
    S_d = nc.dram_tensor("S", [HH, K, W], BF16, kind="ExternalInput").ap()
    P4_d = nc.dram_tensor("P4", [NPAIR, 8, W], BF16, kind="ExternalInput").ap()
    # all band constants packed in one tensor -> a single startup DMA:
    # cols 0:128 BR | 128:188 BR[0:64,64:124] at rows 64.. | 188:192 tail
    # band at rows 0:8 | 192:704 BW
    C_d = nc.dram_tensor("C", [128, 704], BF16, kind="ExternalInput").ap()
    R_d = nc.dram_tensor("R", [NPAIR, HH, W], BF16, kind="ExternalOutput").ap()

    with tile.TileContext(nc) as tc, ExitStack() as ctx:
        const_p = ctx.enter_context(tc.tile_pool(name="const", bufs=1))
        sp_p = ctx.enter_context(tc.tile_pool(name="sp", bufs=1))
        t_p = ctx.enter_context(tc.tile_pool(name="tprod", bufs=t_bufs))
        tp_p = ctx.enter_context(tc.tile_pool(name="tpool", bufs=1))
        i1_p = ctx.enter_context(tc.tile_pool(name="i1", bufs=1))
        r_p = ctx.enter_context(tc.tile_pool(name="rout", bufs=r_bufs))
        ps1_p = ctx.enter_context(
            tc.tile_pool(name="ps1", bufs=ps1_bufs, space="PSUM"))
        ps2_p = ctx.enter_context(
            tc.tile_pool(name="ps2", bufs=ps2_bufs, space="PSUM"))

        ncopy = 0

        def copy_psum(dst, src):
            nonlocal ncopy
            if split_copies:
                # halve latency: two engines copy half each
                h = src.shape[1] // 2
                if ncopy % 2 == 0:
                    nc.vector.tensor_copy(dst[:, 0:h], src[:, 0:h])
                    nc.scalar.copy(dst[:, h:], src[:, h:])
                else:
                    nc.scalar.copy(dst[:, 0:h], src[:, 0:h])
                    nc.vector.tensor_copy(dst[:, h:], src[:, h:])
                ncopy += 1
                return
            if ncopy % dve_sched[0] in dve_sched[1]:
                nc.vector.tensor_copy(dst, src)
            else:
                nc.scalar.copy(dst, src)
            ncopy += 1

        cst = const_p.tile([128, 704], BF16)
        sp = sp_p.tile([HH, K, W], BF16)
        p4_p = ctx.enter_context(tc.tile_pool(name="p4", bufs=6))
        nc.sync.dma_start(sp[:, 0:8, :], S_d[:, 0:8, :])
        nc.sync.dma_start(cst[:], C_d)
        nc.sync.dma_start(sp[:, 8:16, :], S_d[:, 8:16, :])
        br = cst[:, 0:128]
        br_mid = cst[:, 128:188]
        br8 = cst[0:8, 188:192]
        bw = cst[:, 192:704]


        # i1[w_local, ch, chunk, h]: stage-1 output; chunk X covers w 128X..+128
        i1 = i1_p.tile([128, NPAIR, 2, HH], BF16, name="i1")

        def products(oc, mul, pool):
            T = pool.tile([HH, 8, W], BF16,
                          name=f"T{oc}" if pool is tp_p else "T")
            for (j0, k, l0, nl) in _ksegs_in_octet(oc):
                in0 = sp[:, k, :].unsqueeze(1).broadcast_to([HH, nl, W])
                mul(T[:, j0:j0 + nl, :], in0, sp[:, l0:l0 + nl, :])
            return T

        # Pool pre-pass: spread product octets, in consumption order
        prod_T = {}
        for oc in pool_octs:
            prod_T[oc] = products(oc, nc.gpsimd.tensor_mul, tp_p)

        p4_T = {}

        def prefetch_products(ocp):
            """Emit DVE products and tail-product DMAs for group ocp ahead of
            older copies in the DVE queue (products gate PE)."""
            for oi in range(2 if ocp < 8 else 1):
                oc = ocp * 2 + oi
                if oc >= NOCT:
                    continue
                p4o = p4_p.tile([8, 8, W], BF16, name="p4o")
                nc.sync.dma_start(p4o[:], P4_d[oc * 8:(oc + 1) * 8])
                p4_T[oc] = p4o
                if oc not in prod_T:
                    prod_T[oc] = products(oc, nc.vector.tensor_mul, t_p)

        bw0 = bw[:, 0:256]
        bw1 = bw[:, 256:512]
        brt0 = br[0:68, 0:64]
        br_midv = br_mid[64:128, :]
        br8v = br8

        def stage1(ocp):
            nocts = 2 if ocp < 8 else 1
            for oi in range(nocts):
                oc = ocp * 2 + oi
                T = prod_T[oc]
                p4o = p4_T[oc]
                for rt in range(2):
                    ps1 = ps1_p.tile([128, 1024], F32, name="ps1")
                    for j in range(8):
                        for c in range(2):
                            o = j * 128 + c * 64
                            wsl = slice(c * 128, (c + 1) * 128)
                            if rt == 0:
                                nc.tensor.matmul(ps1[:, o:o + 64],
                                                 T[0:68, j, wsl], brt0,
                                                 start=True, stop=True)
                            else:
                                # outs 64..123 from product rows 64..127,
                                # outs 124..127 from host tail products
                                nc.tensor.matmul(ps1[:, o:o + 60],
                                                 T[64:128, j, wsl], br_midv,
                                                 start=True, stop=True)
                                nc.tensor.matmul(ps1[:, o + 60:o + 64],
                                                 p4o[:, j, wsl], br8v,
                                                 start=True, stop=True)
                    copy_psum(
                        i1[:, oc * 8:(oc + 1) * 8, :, rt * 64:(rt + 1) * 64],
                        ps1[:].rearrange("p (c k h) -> p c k h", c=8, k=2))

        def stage2(ocp):
            # ---- stage 2: col box -> [128 h, 256 w] per channel
            nocts = 2 if ocp < 8 else 1
            c0, nch = ocp * 16, 8 * nocts
            g = s2_grain
            for cq in range(nch // g):
                ps2 = ps2_p.tile([128, 256 * g], F32, name="ps2")
                for ci in range(g):
                    c = c0 + cq * g + ci
                    # w-chunk X only reaches wout [128X-2, 128X+129]; stream
                    # each chunk over just its nonzero band columns.
                    l0, l1 = i1[:, c, 0, :], i1[:, c, 1, :]
                    o = ci * 256
                    nc.tensor.matmul(ps2[:, o:o + 126],
                                     l0, bw0[:, 0:126],
                                     start=True, stop=True)
                    nc.tensor.matmul(ps2[:, o + 126:o + 130],
                                     l0, bw0[:, 126:130],
                                     start=True, stop=False)
                    nc.tensor.matmul(ps2[:, o + 126:o + 130],
                                     l1, bw1[:, 126:130],
                                     start=False, stop=True)
                    nc.tensor.matmul(ps2[:, o + 130:o + 256],
                                     l1, bw1[:, 130:256],
                                     start=True, stop=True)
                rsb = r_p.tile([128, g, W], BF16, name="rsb")
                src = ps2[:].rearrange("p (c w) -> p c w", c=g)
                if split_s2:
                    # both engines copy half: halves psum-recycle latency
                    nc.vector.tensor_copy(rsb[:, 0:g // 2, :],
                                          src[:, 0:g // 2])
                    nc.scalar.copy(rsb[:, g // 2:g, :], src[:, g // 2:])
                else:
                    copy_psum(rsb[:], src)
                cb = c0 + cq * g
                dview = R_d[cb:cb + g, :, :].transpose([1, 0, 2])
                nc.sync.dma_start(dview, rsb[:])

        # software-pipeline: keep PE s2_lag groups ahead of stage 2 so
        # stage-2 matmuls never head-block PE while their i1 copies land.
        for ocp in range(s2_lag + 1):
            prefetch_products(ocp)
        for ocp in range(s2_lag):
            stage1(ocp)
        for ocp in range(s2_lag, 9):
            prefetch_products(ocp + 1)
            stage1(ocp)
            stage2(ocp - s2_lag)
        for ocp in range(9 - s2_lag, 9):
            stage2(ocp)

    nc.compile()
    return nc


_NC_CACHE = {}


def _get_nc():
    if "nc" not in _NC_CACHE:
        _NC_CACHE["nc"] = _build_kernel()
    return _NC_CACHE["nc"]


def _prep_in_maps(S):
    S = np.asarray(S, dtype=np.float32)
    np_bf16 = mybir.dt.np(BF16)
    bwm = _build_bw()
    csts = []
    for h in range(2):
        brm = _build_br(h)
        cm = np.zeros((128, 704), dtype=np.float32)
        cm[0:68, 0:128] = brm
        cm[64:128, 128:188] = brm[0:64, 64:124]
        cm[0:8, 188:192] = brm[60:68, 124:128]
        cm[:, 192:704] = bwm
        csts.append(np.ascontiguousarray(cm).astype(np_bf16))
    iu, il = np.triu_indices(K)
    Ss = S * np.float32(0.2)
    in_maps = []
    for b in range(B):
        for half in range(2):
            hbase = half * HH
            rows = np.clip(np.arange(hbase - 2, hbase + 130), 0, H - 1)
            shard = Ss[b][:, rows, :]                      # [K, 132, 256]
            s128 = np.ascontiguousarray(
                shard[:, 0:HH, :].transpose(1, 0, 2)).astype(np_bf16)
            # host products for tail rows 124..131 (feed h-outs 124..127),
            # laid out [(oct, r), ch_in_oct, w] to match the device tiles
            tail = shard[:, 124:SR, :]                     # [K, 8, 256]
            p4 = (tail[iu] * tail[il]).reshape(NOCT, 8, 8, W)
            p4 = np.ascontiguousarray(
                p4.transpose(0, 2, 1, 3).reshape(NPAIR, 8, W)).astype(np_bf16)
            in_maps.append({"S": s128, "P4": p4, "C": csts[half]})
    return in_maps


def _box25(x):
    """Separable 5x5 box sum with reflect padding over last two axes."""
    xp = np.pad(x, ((0, 0), (0, 0), (2, 2), (2, 2)), mode="reflect")
    yh = xp[:, :, 0:H, :].copy()
    for i in range(1, 5):
        yh += xp[:, :, i:i + H, :]
    y = yh[:, :, :, 0:W].copy()
    for j in range(1, 5):
        y += yh[:, :, :, j:j + W]
    return y


def _assemble(results, S):
    iu, il = np.triu_indices(K)            # same order as device channels
    mu = _box25(np.asarray(S, np.float32)) * np.float32(1.0 / 25.0)
    out = np.empty((B, H, W, K, K), dtype=np.float32)
    for i in range(8):
        b, half = divmod(i, 2)
        hs = slice(half * HH, (half + 1) * HH)
        r = np.asarray(results[i]["R"]).astype(np.float32)   # [136, 128, 256]
        v = r - mu[b, iu, hs, :] * mu[b, il, hs, :]          # [136, 128, 256]
        v = np.moveaxis(v, 0, -1)                            # [128, 256, 136]
        flat = np.empty((HH, W, K * K), dtype=np.float32)
        flat[..., iu * K + il] = v
        flat[..., il * K + iu] = v
        out[b, hs] = flat.reshape(HH, W, K, K)
    return out


def kernel(S):
    """S: [4, 16, 256, 256] float32 -> R: [4, 256, 256, 16, 16] float32."""
    nc = _get_nc()
    in_maps = _prep_in_maps(S)
    res = bass_utils.run_bass_kernel_spmd(nc, in_maps, list(range(8)))
    return _assemble(res.results, S)


# revision 66
# speedup vs baseline: 1.2344x; 1.0635x over previous
"""Trainium2 Bass kernel: per-pixel 5x5-patch channel covariance.

R[b,h,w,k,l] = (1/N) sum_n (p_kn - mu_k)(p_ln - mu_l)   (N=25, reflect pad)

Identity:  R = box5x5(S_k * S_l)/25 - mu_k * mu_l,  mu = box5x5(S)/25.

Device computes ONLY the 136 upper-triangle pair channels box5x5(S_k*S_l)/25
(host pre-scales S by 1/5 so two weight-1 banded box passes give /25).
Host computes mu (cheap separable box in fp32), subtracts mu_k*mu_l, and
mirrors the symmetric lower triangle -- all trivially vectorized numpy.

Per core (shard = one batch x one H-half): products for shard rows 0..127
are computed once on a full 128-partition tile (DVE/Pool); the 4 halo rows
128..131 only feed h-outs 124..127, so their products are precomputed on
the host (tiny) and folded in via an N=4 accumulating matmul.

  products (DVE 2x / Pool)
    -> stage-1 row-box banded matmuls (TensorE, psum [w, (8c,2chunk,64h)])
    -> psum->sbuf copies into i1[w, ch, chunk, h] (Act/DVE)
    -> stage-2 col-box matmuls lhsT=i1[:,c,k,:], rhs=BW chunk (psum [h, w])
    -> psum->sbuf copies (Act/DVE)
    -> DMA out channel-major [136, 128, 256] (512B descriptors)

Sharding: 8 cores = 4 batches x 2 H-halves.  Fully data parallel.
"""
import sys

sys.path.insert(0, "/opt/trn_rl_repo")

from contextlib import ExitStack

import numpy as np

import concourse.bacc as bacc
import concourse.mybir as mybir
import concourse.tile as tile
from concourse import bass_utils

B, K, H, W = 4, 16, 256, 256
HH = 128           # output rows per core
SR = 132           # shard rows (128 + 2 halo each side, edge-clamped)
NPAIR = K * (K + 1) // 2   # 136 upper-triangle channels
NOCT = NPAIR // 8          # 17 channel octets
F32 = mybir.dt.float32
BF16 = mybir.dt.bfloat16

# Pool (GPSIMD) cannot read PSUM, so psum->sbuf copies go Act/DVE; Pool
# takes these product octets (emitted as a pre-pass, consumption order).
POOL_OCTS = [2, 4, 6, 8, 10, 12, 15]


def _reflect_idx(i, n):
    if i < 0:
        return -i
    if i >= n:
        return 2 * (n - 1) - i
    return i


def _build_bw():
    """[128, 512] col-box weights, reflect folded: [:, c*256:(c+1)*256] =
    M[c*128:(c+1)*128, :] where M[w_src, w_out] is the 256x256 band."""
    M = np.zeros((W, W), dtype=np.float32)
    for w in range(W):
        for j in range(5):
            M[_reflect_idx(w - 2 + j, W), w] += 1.0
    out = np.zeros((128, 512), dtype=np.float32)
    out[:, 0:256] = M[0:128, :]
    out[:, 256:512] = M[128:256, :]
    return out


def _build_br(half):
    """[68, 128] row-box weights: cols rt*64+hl; rows shard-local within rt."""
    hbase = half * HH
    M = np.zeros((68, 128), dtype=np.float32)
    for rt in range(2):
        for hl in range(64):
            hg = hbase + rt * 64 + hl
            for i in range(5):
                r = _reflect_idx(hg - 2 + i, H)
                j = r + 2 - hbase          # canonical shard row
                M[j - rt * 64, rt * 64 + hl] += 1.0
    return M


def _ksegs_in_octet(oct_idx):
    """Pair channels 0..135 in (k outer, l=k..15) order. For channel octet
    [oct*8, oct*8+8) return (j0, k, l0, nl): local offset, k, first l, count."""
    lo, hi = oct_idx * 8, oct_idx * 8 + 8
    segs = []
    p = 0
    for k in range(K):
        n = K - k
        s, e = p, p + n
        a, b = max(lo, s), min(hi, e)
        if a < b:
            segs.append((a - lo, k, k + (a - s), b - a))
        p += n
    return segs


def _build_kernel(pool_octs=None, dve_sched=(5, (1, 3)),
                  ps1_bufs=2, ps2_bufs=2, r_bufs=8, t_bufs=8, s2_grain=4,
                  split_copies=False, s2_lag=1, split_s2=False):
    if pool_octs is None:
        pool_octs = POOL_OCTS
    nc = bacc.Bacc("TRN2", target_bir_lowering=False, debug=False)
    S_d = nc.dram_tensor("S", [HH, K, W], BF16, kind="ExternalInput").ap()
    P4_d = nc.dram_tensor("P4", [NPAIR, 8, W], BF16, kind="ExternalInput").ap()
    # all band constants packed in one tensor -> a single startup DMA:
    # cols 0:128 BR | 128:188 BR[0:64,64:124] at rows 64.. | 188:192 tail
    # band at rows 0:8 | 192:704 BW
    C_d = nc.dram_tensor("C", [128, 704], BF16, kind="ExternalInput").ap()
    R_d = nc.dram_tensor("R", [NPAIR, HH, W], BF16, kind="ExternalOutput").ap()

    with tile.TileContext(nc) as tc, ExitStack() as ctx:
        const_p = ctx.enter_context(tc.tile_pool(name="const", bufs=1))
        sp_p = ctx.enter_context(tc.tile_pool(name="sp", bufs=1))
        t_p = ctx.enter_context(tc.tile_pool(name="tprod", bufs=t_bufs))
        tp_p = ctx.enter_context(tc.tile_pool(name="tpool", bufs=1))
        i1_p = ctx.enter_context(tc.tile_pool(name="i1", bufs=1))
        r_p = ctx.enter_context(tc.tile_pool(name="rout", bufs=r_bufs))
        ps1_p = ctx.enter_context(
            tc.tile_pool(name="ps1", bufs=ps1_bufs, space="PSUM"))
        ps2_p = ctx.enter_context(
            tc.tile_pool(name="ps2", bufs=ps2_bufs, space="PSUM"))

        ncopy = 0

        def copy_psum(dst, src):
            nonlocal ncopy
            if split_copies:
                # halve latency: two engines copy half each
                h = src.shape[1] // 2
                if ncopy % 2 == 0:
                    nc.vector.tensor_copy(dst[:, 0:h], src[:, 0:h])
                    nc.scalar.copy(dst[:, h:], src[:, h:])
                else:
                    nc.scalar.copy(dst[:, 0:h], src[:, 0:h])
                    nc.vector.tensor_copy(dst[:, h:], src[:, h:])
                ncopy += 1
                return
            if ncopy % dve_sched[0] in dve_sched[1]:
                nc.vector.tensor_copy(dst, src)
            else:
                nc.scalar.copy(dst, src)
            ncopy += 1

        cst = const_p.tile([128, 704], BF16)
        sp = sp_p.tile([HH, K, W], BF16)
        p4_p = ctx.enter_context(tc.tile_pool(name="p4", bufs=6))
        nc.sync.dma_start(sp[:, 0:8, :], S_d[:, 0:8, :])
        nc.sync.dma_start(cst[:], C_d)
        nc.sync.dma_start(sp[:, 8:16, :], S_d[:, 8:16, :])
        br = cst[:, 0:128]
        br_mid = cst[:, 128:188]
        br8 = cst[0:8, 188:192]
        bw = cst[:, 192:704]


        # i1[w_local, ch, chunk, h]: stage-1 output; chunk X covers w 128X..+128
        i1 = i1_p.tile([128, NPAIR, 2, HH], BF16, name="i1")

        def products(oc, mul, pool):
            T = pool.tile([HH, 8, W], BF16,
                          name=f"T{oc}" if pool is tp_p else "T")
            for (j0, k, l0, nl) in _ksegs_in_octet(oc):
                in0 = sp[:, k, :].unsqueeze(1).broadcast_to([HH, nl, W])
                mul(T[:, j0:j0 + nl, :], in0, sp[:, l0:l0 + nl, :])
            return T

        # Pool pre-pass: spread product octets, in consumption order
        prod_T = {}
        for oc in pool_octs:
            prod_T[oc] = products(oc, nc.gpsimd.tensor_mul, tp_p)

        p4_T = {}

        def prefetch_products(ocp):
            """Emit DVE products and tail-product DMAs for group ocp ahead of
            older copies in the DVE queue (products gate PE)."""
            for oi in range(2 if ocp < 8 else 1):
                oc = ocp * 2 + oi
                if oc >= NOCT:
                    continue
                p4o = p4_p.tile([8, 8, W], BF16, name="p4o")
                nc.sync.dma_start(p4o[:], P4_d[oc * 8:(oc + 1) * 8])
                p4_T[oc] = p4o
                if oc not in prod_T:
                    prod_T[oc] = products(oc, nc.vector.tensor_mul, t_p)

        bw0 = bw[:, 0:256]
        bw1 = bw[:, 256:512]
        brt0 = br[0:68, 0:64]
        br_midv = br_mid[64:128, :]
        br8v = br8

        def stage1(ocp):
            nocts = 2 if ocp < 8 else 1
            for oi in range(nocts):
                oc = ocp * 2 + oi
                T = prod_T[oc]
                p4o = p4_T[oc]
                for rt in range(2):
                    ps1 = ps1_p.tile([128, 1024], F32, name="ps1")
                    for j in range(8):
                        for c in range(2):
                            o = j * 128 + c * 64
                            wsl = slice(c * 128, (c + 1) * 128)
                            if rt == 0:
                                nc.tensor.matmul(ps1[:, o:o + 64],
                                                 T[0:68, j, wsl], brt0,
                                                 start=True, stop=True)
                            else:
                                # outs 64..123 from product rows 64..127,
                                # outs 124..127 from host tail products
                                nc.tensor.matmul(ps1[:, o:o + 60],
                                                 T[64:128, j, wsl], br_midv,
                                                 start=True, stop=True)
                                nc.tensor.matmul(ps1[:, o + 60:o + 64],
                                                 p4o[:, j, wsl], br8v,
                                                 start=True, stop=True)
                    copy_psum(
                        i1[:, oc * 8:(oc + 1) * 8, :, rt * 64:(rt + 1) * 64],
                        ps1[:].rearrange("p (c k h) -> p c k h", c=8, k=2))

        def stage2(ocp):
            # ---- stage 2: col box -> [128 h, 256 w] per channel
            nocts = 2 if ocp < 8 else 1
            c0, nch = ocp * 16, 8 * nocts
            g = s2_grain
            for cq in range(nch // g):
                ps2 = ps2_p.tile([128, 256 * g], F32, name="ps2")
                for ci in range(g):
                    c = c0 + cq * g + ci
                    # w-chunk X only reaches wout [128X-2, 128X+129]; stream
                    # each chunk over just its nonzero band columns.
                    l0, l1 = i1[:, c, 0, :], i1[:, c, 1, :]
                    o = ci * 256
                    nc.tensor.matmul(ps2[:, o:o + 126],
                                     l0, bw0[:, 0:126],
                                     start=True, stop=True)
                    nc.tensor.matmul(ps2[:, o + 126:o + 130],
                                     l0, bw0[:, 126:130],
                                     start=True, stop=False)
                    nc.tensor.matmul(ps2[:, o + 126:o + 130],
                                     l1, bw1[:, 126:130],
                                     start=False, stop=True)
                    nc.tensor.matmul(ps2[:, o + 130:o + 256],
                                     l1, bw1[:, 130:256],
                                     start=True, stop=True)
                rsb = r_p.tile([128, g, W], BF16, name="rsb")
                src = ps2[:].rearrange("p (c w) -> p c w", c=g)
                if split_s2:
                    # both engines copy half: halves psum-recycle latency
                    nc.vector.tensor_copy(rsb[:, 0:g // 2, :],
                                          src[:, 0:g // 2])
                    nc.scalar.copy(rsb[:, g // 2:g, :], src[:, g // 2:])
                else:
                    copy_psum(rsb[:], src)
                cb = c0 + cq * g
                dview = R_d[cb:cb + g, :, :].transpose([1, 0, 2])
                nc.sync.dma_start(dview, rsb[:])

        # software-pipeline: keep PE s2_lag groups ahead of stage 2 so
        # stage-2 matmuls never head-block PE while their i1 copies land.
        for ocp in range(s2_lag + 1):
            prefetch_products(ocp)
        for ocp in range(s2_lag):
            stage1(ocp)
        for ocp in range(s2_lag, 9):
            prefetch_products(ocp + 1)
            stage1(ocp)
            stage2(ocp - s2_lag)
        for ocp in range(9 - s2_lag, 9):
            stage2(ocp)

    nc.compile()
    return nc


_NC_CACHE = {}


def _get_nc():
    if "nc" not in _NC_CACHE:
        _NC_CACHE["nc"] = _build_kernel()
    return _NC_CACHE["nc"]


def _prep_in_maps(S):
    S = np.asarray(S, dtype=np.float32)
    np_bf16 = mybir.dt.np(BF16)
    bwm = _build_bw()
    csts = []
    for h in range(2):
        brm = _build_br(h)
        cm = np.zeros((128, 704), dtype=np.float32)
        cm[0:68, 0:128] = brm
        cm[64:128, 128:188] = brm[0:64, 64:124]
        cm[0:8, 188:192] = brm[60:68, 124:128]
        cm[:, 192:704] = bwm
        csts.append(np.ascontiguousarray(cm).astype(np_bf16))
    iu, il = np.triu_indices(K)
    Ss = S * np.float32(0.2)
    in_maps = []
    for b in range(B):
        for half in range(2):
            hbase = half * HH
            rows = np.clip(np.arange(hbase - 2, hbase + 130), 0, H - 1)
            shard = Ss[b][:, rows, :]                      # [K, 132, 256]
            s128 = np.ascontiguousarray(
                shard[:, 0:HH, :].transpose(1, 0, 2)).astype(np_bf16)
            # host products for tail rows 124..131 (feed h-outs 124..127),
            # laid out [(oct, r), ch_in_oct, w] to match the device tiles
            tail = shard[:, 124:SR, :]                     # [K, 8, 256]
            p4 = (tail[iu] * tail[il]).reshape(NOCT, 8, 8, W)
            p4 = np.ascontiguousarray(
                p4.transpose(0, 2, 1, 3).reshape(NPAIR, 8, W)).astype(np_bf16)
            in_maps.append({"S": s128, "P4": p4, "C": csts[half]})
    return in_maps


def _box25(x):
    """Separable 5x5 box sum with reflect padding over last two axes."""
    xp = np.pad(x, ((0, 0), (0, 0), (2, 2), (2, 2)), mode="reflect")
    yh = xp[:, :, 0:H, :].copy()
    for i in range(1, 5):
        yh += xp[:, :, i:i + H, :]
    y = yh[:, :, :, 0:W].copy()
    for j in range(1, 5):
        y += yh[:, :, :, j:j + W]
    return y


def _assemble(results, S):
    iu, il = np.triu_indices(K)            # same order as device channels
    mu = _box25(np.asarray(S, np.float32)) * np.float32(1.0 / 25.0)
    out = np.empty((B, H, W, K, K), dtype=np.float32)
    for i in range(8):
        b, half = divmod(i, 2)
        hs = slice(half * HH, (half + 1) * HH)
        r = np.asarray(results[i]["R"]).astype(np.float32)   # [136, 128, 256]
        v = r - mu[b, iu, hs, :] * mu[b, il, hs, :]          # [136, 128, 256]
        v = np.moveaxis(v, 0, -1)                            # [128, 256, 136]
        flat = np.empty((HH, W, K * K), dtype=np.float32)
        flat[..., iu * K + il] = v
        flat[..., il * K + iu] = v
        out[b, hs] = flat.reshape(HH, W, K, K)
    return out


def kernel(S):
    """S: [4, 16, 256, 256] float32 -> R: [4, 256, 256, 16, 16] float32."""
    nc = _get_nc()
    in_maps = _prep_in_maps(S)
    res = bass_utils.run_bass_kernel_spmd(nc, in_maps, list(range(8)))
    return _assemble(res.results, S)
